# revision 1
# baseline (speedup 1.0000x reference)
"""Trainium2 distributed kernel for nn_CPAM_Module (CPAM attention block).

Math collapse (verified exact vs reference, ~2.6e-8 fro rel err in f64):
  te   = text_flat @ G_w.T + G_b                      (B, C)
  te_flat = te[:, :, None] * l  (rank-1 per batch)  =>
  proj_key / proj_value are rank-1 in n; energy[b,n,m] = s[b,n]*l[m] + const(n)
  softmax over m kills the const =>
  attn[b,n,m] = softmax_m(s[b,n] * l[m])
  s[b,n] = sum_c u[b,c] x[b,c,n] + b_q.kte[b],  u = kte @ W_q, kte = te @ W_k.T
  a[b,n] = (sum_j l_j e^{l_j s}) / (sum_j e^{l_j s})
  out    = gamma * (vte[b,c] * a[b,n] + b_v[c]) + x,  vte = te @ W_v.T

Sharding: contraction (TXT=153600) split 8 ways for the big G matmul;
ReduceScatter of te (bf16) hands each core its 32 batches; epilogue is
batch-parallel. x/out traffic is B-sharded (25.7 MB each per core).
"""

import sys

sys.path.insert(0, "/opt/trn_rl_repo")

import numpy as np
import ml_dtypes

from concourse import bass, bacc, mybir, tile
from concourse.bass_utils import run_bass_kernel_spmd

F32 = mybir.dt.float32
BF16 = mybir.dt.bfloat16
FP8 = mybir.dt.float8e4
GW_SCALE = 256.0
AF = mybir.ActivationFunctionType
ALU = mybir.AluOpType

N_CORES = 8
B, C, H, W = 256, 1024, 14, 14
N = H * W  # 196
C8 = 128
TXT = 150 * 1024
KSH = TXT // N_CORES  # 19200 txt-contraction shard per core
NK = KSH // 128  # 150 k-tiles
BL = B // N_CORES  # 32 local batches
CT = C // 128  # 8 c tiles
JT = 98  # j-tile (196 = 2*98)


def build(gamma: float, skip_gb: bool, skip_bq: bool, skip_bv: bool, single: bool = False, repeat: int = 1, loop_n: int = 0, part: str = 'all'):
    # single=True builds a 1-core variant with the ReduceScatter replaced by a
    # local DMA (same bytes landing in te_rs) so TimelineSim can model it.
    nc = bacc.Bacc(
        "TRN2",
        target_bir_lowering=False,
        debug=False,
        num_devices=1 if single else N_CORES,
    )

    text_t = nc.dram_tensor("text_t", [128, NK * B], FP8, kind="ExternalInput")
    g_wt = nc.dram_tensor("g_wt", [128, NK * C], FP8, kind="ExternalInput")
    xs = nc.dram_tensor("xs", [BL, 128, CT * N], BF16, kind="ExternalInput")
    w_vt = nc.dram_tensor("w_vt", [128, CT * C], BF16, kind="ExternalInput")
    w_kt = nc.dram_tensor("w_kt", [128, CT * C8], BF16, kind="ExternalInput")
    w_q = nc.dram_tensor("w_q", [C8, C], BF16, kind="ExternalInput")
    lrow = nc.dram_tensor("lrow", [1, N], F32, kind="ExternalInput")
    l_bc = nc.dram_tensor("l_bc", [128, N], BF16, kind="ExternalInput")
    lw = nc.dram_tensor("lw", [N, 2], BF16, kind="ExternalInput")
    g_b = nc.dram_tensor("g_b", [C8, CT], F32, kind="ExternalInput")
    b_q = nc.dram_tensor("b_q", [C8, 1], BF16, kind="ExternalInput")
    gbv = nc.dram_tensor("gbv", [C8, CT], F32, kind="ExternalInput")
    out = nc.dram_tensor("out", [BL, 128, CT * N], BF16, kind="ExternalOutput")

    with tile.TileContext(nc) as tc:
        with (
            tc.tile_pool(name="const", bufs=1) as const,
            tc.tile_pool(name="dram", bufs=1, space="DRAM") as dram,
        ):
            # Constants
            lbc_sb = const.tile([128, N], BF16, tag="lbc")
            nc.sync.dma_start(lbc_sb[:], l_bc[:, :])
            lw0 = const.tile([JT, 2], BF16, tag="lw0")
            lw1 = const.tile([JT, 2], BF16, tag="lw1")
            nc.sync.dma_start(lw0[:], lw[0:JT, :])
            nc.sync.dma_start(lw1[:], lw[JT : 2 * JT, :])
            wvt_sb = const.tile([128, CT, C], BF16, tag="wvt")
            nc.scalar.dma_start(wvt_sb[:].opt(), w_vt[:, :])
            wkt_sb = const.tile([128, CT, C8], BF16, tag="wkt")
            nc.scalar.dma_start(wkt_sb[:].opt(), w_kt[:, :])
            wq_sb = const.tile([C8, C], BF16, tag="wq")
            nc.sync.dma_start(wq_sb[:], w_q[:, :])
            if not skip_gb:
                gb_sb = const.tile([C8, CT], F32, tag="gb")
                nc.sync.dma_start(gb_sb[:], g_b[:, :])
            if not skip_bq:
                bq_sb = const.tile([C8, 1], BF16, tag="bq")
                nc.sync.dma_start(bq_sb[:], b_q[:, :])
            if not skip_bv:
                gbv_sb = const.tile([C8, CT], F32, tag="gbv")
                nc.sync.dma_start(gbv_sb[:], gbv[:, :])
            ones128 = const.tile([1, C8], F32, tag="ones128")
            nc.vector.memset(ones128[:], 1.0)

            te_full = dram.tile([B, C], BF16)
            te_rs = dram.tile([BL, C], BF16)

            if loop_n:
                assert single, "hardware loop timing mode is single-core only"
                loop_cm = tc.For_i(0, loop_n, 1)
                loop_cm.__enter__()
            for _rep in range(repeat):
                if part in ("all", "g"):
                    # ---- Phase 1: G matmul, te_partial[b, c] over local txt shard ----
                    with (
                        tc.tile_pool(name=f"gpsum{_rep}", bufs=4, space="PSUM") as gp,
                        tc.tile_pool(name=f"tl{_rep}", bufs=3) as tlp,
                        tc.tile_pool(name=f"gw{_rep}", bufs=3) as gwp,
                        tc.tile_pool(name=f"tesb{_rep}", bufs=4) as tesb,
                    ):
                        pt = [
                            [gp.tile([128, 512], F32, tag="gp", name=f"gp{_rep}_{m}{n2}") for n2 in range(2)]
                            for m in range(2)
                        ]
                        KB = 10  # k-tiles per DMA batch (150 = 15 * 10)
                        NPAIR = NK // 2
                        for g in range(NK // KB):
                            tl_t = tlp.tile([128, KB, B], FP8, tag="tl")
                            nc.sync.dma_start(tl_t[:].opt(), text_t[:, g * KB * B : (g + 1) * KB * B])
                            gw_t = gwp.tile([128, KB, C], FP8, tag="gw")
                            nc.sync.dma_start(gw_t[:].opt(), g_wt[:, g * KB * C : (g + 1) * KB * C])
                            for f in range(0, KB, 2):
                                j = (g * KB + f) // 2  # pair index
                                for m in range(2):
                                    for n2 in range(2):
                                        nc.tensor.matmul(
                                            pt[m][n2][:],
                                            tl_t[:, f : f + 2, m * 128 : (m + 1) * 128],
                                            gw_t[:, f : f + 2, n2 * 512 : (n2 + 1) * 512],
                                            start=(j == 0),
                                            stop=(j == NPAIR - 1),
                                            perf_mode=mybir.MatmulPerfMode.DoubleRow,
                                        )
                        for m in range(2):
                            for n2 in range(2):
                                ev = tesb.tile([128, 512], BF16, tag="tesb")
                                nc.scalar.mul(ev[:], pt[m][n2][:], 1.0 / GW_SCALE)
                                nc.sync.dma_start(
                                    te_full[m * 128 : (m + 1) * 128, n2 * 512 : (n2 + 1) * 512],
                                    ev[:],
                                )

                    # ---- Phase 2: ReduceScatter -> local te (32, 1024) bf16 ----
                    if single:
                        nc.sync.dma_start(te_rs[:, :], te_full[0:BL, :])
                    else:
                        nc.gpsimd.collective_compute(
                            "ReduceScatter",
                            ALU.add,
                            replica_groups=[list(range(N_CORES))],
                            ins=[te_full.opt()],
                            outs=[te_rs.opt()],
                        )

                if part in ("all", "epi"):
                    # ---- Phase 3: epilogue prep: teT, kteT, uT, gvteT ----
                    teT_sb = const.tile([128, CT, BL], BF16, tag="teT")
                    for t in range(CT):
                        nc.scalar.dma_start(
                            teT_sb[:, t, :],
                            te_rs.rearrange("b (t p) -> p t b", p=128)[:, t, :].opt(),
                        )
                    if not skip_gb:
                        for t in range(CT):
                            nc.vector.tensor_scalar_add(
                                teT_sb[:, t, :], teT_sb[:, t, :], gb_sb[:, t : t + 1]
                            )

                    uT_sb = const.tile([128, CT, BL], F32, tag="uT")
                    gvteT_sb = const.tile([128, CT, BL], F32, tag="gvteT")
                    bqd_row = const.tile([1, BL], F32, tag="bqd") if not skip_bq else None

                    with (
                        tc.tile_pool(name=f"ppsum{_rep}", bufs=2, space="PSUM") as pp,
                        tc.tile_pool(name=f"psmall{_rep}", bufs=2) as psm,
                    ):
                        # kteT (q, b) = sum_c W_kT[c, q] * teT[c, b]
                        kteT_ps = pp.tile([C8, BL], F32, tag="pp")
                        for t in range(CT):
                            nc.tensor.matmul(
                                kteT_ps[:],
                                wkt_sb[:, t, :],
                                teT_sb[:, t, :],
                                start=(t == 0),
                                stop=(t == CT - 1),
                            )
                        kteT_sb = psm.tile([C8, BL], BF16, tag="kteT")
                        nc.scalar.copy(kteT_sb[:], kteT_ps[:])

                        # uT (c, b) = sum_q W_q[q, c] * kteT[q, b]
                        for t in range(CT):
                            u_ps = pp.tile([128, BL], F32, tag="pp")
                            nc.tensor.matmul(
                                u_ps[:],
                                wq_sb[:, t * 128 : (t + 1) * 128],
                                kteT_sb[:],
                                start=True,
                                stop=True,
                            )
                            nc.scalar.copy(uT_sb[:, t, :], u_ps[:])

                        # bqdot[b] = sum_q kteT[q, b] * b_q[q]
                        if not skip_bq:
                            bq_ps = pp.tile([BL, 1], F32, tag="ppbq")
                            nc.tensor.matmul(bq_ps[:], kteT_sb[:], bq_sb[:], start=True, stop=True)
                            bqd_col = psm.tile([BL, 1], F32, tag="bqdc")
                            nc.scalar.copy(bqd_col[:], bq_ps[:])
                            nc.sync.dma_start(bqd_row[:].rearrange("o b -> o b 1"), bqd_col[:])

                        # gvteT (c', b) = gamma * sum_c W_vT[c, c'] * teT[c, b]
                        for mt in range(CT):
                            v_ps = pp.tile([128, BL], F32, tag="ppv")
                            for kt in range(CT):
                                nc.tensor.matmul(
                                    v_ps[:],
                                    wvt_sb[:, kt, mt * 128 : (mt + 1) * 128],
                                    teT_sb[:, kt, :],
                                    start=(kt == 0),
                                    stop=(kt == CT - 1),
                                )
                            nc.scalar.mul(gvteT_sb[:, mt, :], v_ps[:], float(gamma))

                    # ---- Phase 4: per-batch attention epilogue ----
                    # Wave-of-4 structure: all ACT Copy-class ops of a wave are
                    # emitted together, then the wave's Exps, then the previous
                    # wave's output Copies — ACT pays ~2 function-table switches
                    # per wave instead of 2 per batch.
                    with (
                        tc.tile_pool(name=f"xp{_rep}", bufs=32) as xp,
                        tc.tile_pool(name=f"op{_rep}", bufs=6) as op,
                        tc.tile_pool(name=f"esb{_rep}", bufs=4) as ep,
                        tc.tile_pool(name=f"small{_rep}", bufs=6) as sm,
                        tc.tile_pool(name=f"ps_z{_rep}", bufs=2, space="PSUM") as ps_z,
                        tc.tile_pool(name=f"ps_dn{_rep}", bufs=2, space="PSUM") as ps_dn,
                        tc.tile_pool(name=f"ps_ab{_rep}", bufs=2, space="PSUM") as ps_ab,
                    ):
                        WAVE = 4
                        st = {}

                        def front_a(b):
                            # x load + y[p,n] = sum_t uT[p,t,b] x[p,t,n]
                            xb = xp.tile([128, CT, N], BF16, tag="xb", name=f"xb{_rep}_{b}")
                            nc.gpsimd.dma_start(xb[:].opt(), xs[b].opt())
                            y_sb = sm.tile([128, N], BF16, tag="y")
                            nc.vector.tensor_scalar_mul(
                                y_sb[:], xb[:, 0, :], uT_sb[:, 0, b : b + 1]
                            )
                            for t in (1, 2, 3, 7):
                                nc.vector.scalar_tensor_tensor(
                                    y_sb[:], xb[:, t, :], uT_sb[:, t, b : b + 1], y_sb[:],
                                    ALU.mult, ALU.add,
                                )
                            tmp = sm.tile([128, 3, N], BF16, tag="ytmp", name=f"ytmp{_rep}_{b}")
                            for t in (4, 5, 6):
                                nc.scalar.activation(
                                    tmp[:, t - 4, :], xb[:, t, :], AF.Copy,
                                    scale=uT_sb[:, t, b : b + 1],
                                )
                            nc.gpsimd.tensor_add(tmp[:, 0, :], tmp[:, 0, :], tmp[:, 1, :])
                            nc.gpsimd.tensor_add(tmp[:, 0, :], tmp[:, 0, :], tmp[:, 2, :])
                            nc.vector.tensor_add(y_sb[:], y_sb[:], tmp[:, 0, :])
                            if not skip_bq:
                                nc.vector.tensor_scalar_add(
                                    y_sb[:, 0:1], y_sb[:, 0:1], bqd_row[0:1, b : b + 1]
                                )
                            st[b] = (xb, y_sb)

                        def front_b(b):
                            # Z = l_bcast^T.y ; E = exp(Z); den/num; a = num/den
                            xb, y_sb = st[b]
                            den_ps = ps_dn.tile([1, N], F32, tag="den")
                            num_ps = ps_dn.tile([1, N], F32, tag="num")
                            for jt in range(2):
                                z_ps = ps_z.tile([JT, N], F32, tag="z")
                                nc.tensor.matmul(
                                    z_ps[:],
                                    lbc_sb[:, jt * JT : (jt + 1) * JT],
                                    y_sb[:],
                                    start=True,
                                    stop=True,
                                )
                                e_sb = ep.tile([JT, N], BF16, tag="e")
                                nc.scalar.activation(e_sb[:], z_ps[:], AF.Exp)
                                lw_t = lw0 if jt == 0 else lw1
                                nc.tensor.matmul(
                                    den_ps[:], lw_t[:, 0:1], e_sb[:],
                                    start=(jt == 0), stop=(jt == 1),
                                )
                                nc.tensor.matmul(
                                    num_ps[:], lw_t[:, 1:2], e_sb[:],
                                    start=(jt == 0), stop=(jt == 1),
                                )
                            dinv = sm.tile([1, N], F32, tag="dinv")
                            nc.vector.reciprocal(dinv[:], den_ps[:])
                            a_sb = sm.tile([1, N], F32, tag="a")
                            nc.vector.tensor_mul(a_sb[:], num_ps[:], dinv[:])
                            st[b] = (xb, a_sb)

                        def back(b):
                            # out[c,n] = gvteT[c,b]*a[n] (+ g*b_v) + x[c,n]
                            xb, a_sb = st.pop(b)
                            ab_ps = ps_ab.tile([128, N], F32, tag="ab")
                            nc.tensor.matmul(
                                ab_ps[:], ones128[:], a_sb[:], start=True, stop=True
                            )
                            ob = op.tile([128, CT, N], BF16, tag="ob")
                            for t in range(5):
                                nc.vector.scalar_tensor_tensor(
                                    ob[:, t, :],
                                    ab_ps[:],
                                    gvteT_sb[:, t, b : b + 1],
                                    xb[:, t, :],
                                    ALU.mult,
                                    ALU.add,
                                )
                            for t in range(5, CT):
                                nc.scalar.activation(
                                    ob[:, t, :], ab_ps[:], AF.Copy,
                                    scale=gvteT_sb[:, t, b : b + 1],
                                )
                            nc.gpsimd.tensor_add(
                                ob[:, 5:CT, :], ob[:, 5:CT, :], xb[:, 5:CT, :]
                            )
                            if not skip_bv:
                                for t in range(CT):
                                    nc.vector.tensor_scalar_add(
                                        ob[:, t, :], ob[:, t, :], gbv_sb[:, t : t + 1]
                                    )
                            nc.sync.dma_start(out[b].opt(), ob[:].opt())

                        waves = [list(range(w, min(w + WAVE, BL))) for w in range(0, BL, WAVE)]
                        for wi, wave in enumerate(waves):
                            for b in wave:
                                front_a(b)
                            for b in wave:
                                front_b(b)
                            if wi >= 1:
                                for b in waves[wi - 1]:
                                    back(b)
                        for b in waves[-1]:
                            back(b)
            if loop_n:
                loop_cm.__exit__(None, None, None)

    nc.compile()
    return nc


def _prep_inputs(inputs):
    """Host-side sharding. Returns in_maps for the 8 cores."""
    x = np.ascontiguousarray(inputs["x"], dtype=np.float32).reshape(B, C, N)
    text = np.ascontiguousarray(inputs["text_embed"], dtype=np.float32).reshape(B, -1)
    G_w = np.asarray(inputs["G_w"], dtype=np.float32)
    l = np.asarray(inputs["l"], dtype=np.float32).reshape(1, N)
    W_q = np.asarray(inputs["W_q"], dtype=np.float32)
    W_k = np.asarray(inputs["W_k"], dtype=np.float32)
    W_v = np.asarray(inputs["W_v"], dtype=np.float32)
    b_v = np.asarray(inputs["b_v"], dtype=np.float32)
    b_q = np.asarray(inputs["b_q"], dtype=np.float32)
    G_b = np.asarray(inputs["G_b"], dtype=np.float32)
    gamma = float(np.asarray(inputs["gamma"]).reshape(-1)[0])

    bf = ml_dtypes.bfloat16
    f8 = ml_dtypes.float8_e4m3

    def pretile(a, p=128):
        # (T*p, F) -> (p, T*F): partition-major tiling for contiguous DMA
        tp, f = a.shape
        t = tp // p
        return np.ascontiguousarray(a.reshape(t, p, f).transpose(1, 0, 2).reshape(p, t * f))

    w_vt = pretile(np.ascontiguousarray(W_v.T).astype(bf))
    w_kt = pretile(np.ascontiguousarray(W_k.T).astype(bf))
    w_q = W_q.astype(bf)
    lw = np.stack([np.ones(N, np.float32), l[0]], axis=1)  # (196, 2)
    g_b_t = np.ascontiguousarray(G_b.reshape(CT, C8).T)  # (128, 8)
    gbv = np.ascontiguousarray((gamma * b_v).reshape(CT, C8).T)
    b_q_col = b_q.reshape(C8, 1).astype(bf)

    in_maps = []
    for i in range(N_CORES):
        sl = slice(i * KSH, (i + 1) * KSH)
        in_maps.append(
            {
                "text_t": pretile(np.ascontiguousarray(text[:, sl].T).astype(f8)),
                "g_wt": pretile((np.ascontiguousarray(G_w[:, sl].T) * 256.0).astype(f8)),
                "xs": np.ascontiguousarray(
                    x[i * BL : (i + 1) * BL]
                    .reshape(BL, CT, 128, N)
                    .transpose(0, 2, 1, 3)
                    .reshape(BL, 128, CT * N)
                ).astype(bf),
                "w_vt": w_vt,
                "w_kt": w_kt,
                "w_q": w_q,
                "lrow": l,
                "l_bc": np.ascontiguousarray(np.broadcast_to(l, (128, N))).astype(bf),
                "lw": lw.astype(bf),
                "g_b": g_b_t,
                "b_q": b_q_col,
                "gbv": gbv,
            }
        )
    meta = {
        "gamma": gamma,
        "skip_gb": not np.any(G_b),
        "skip_bq": not np.any(b_q),
        "skip_bv": not np.any(b_v),
    }
    return in_maps, meta


def _run(inputs, trace=False, repeat=1):
    in_maps, meta = _prep_inputs(inputs)
    nc = build(meta["gamma"], meta["skip_gb"], meta["skip_bq"], meta["skip_bv"], repeat=repeat)
    res = run_bass_kernel_spmd(nc, in_maps, core_ids=list(range(N_CORES)), trace=trace)
    outs = [
        res.results[i]["out"]
        .astype(np.float32)
        .reshape(BL, 128, CT, N)
        .transpose(0, 2, 1, 3)
        .reshape(BL, C, N)
        for i in range(N_CORES)
    ]
    full = np.concatenate(outs, axis=0).reshape(B, C, H, W)
    return full, res


def kernel(**inputs) -> np.ndarray:
    full, _ = _run(inputs, trace=False)
    return full


if __name__ == "__main__":
    import reference

    inputs = {k: np.asarray(v) for k, v in reference.setup_inputs().items()}
    got = kernel(**inputs)
    print("out shape:", got.shape, got.dtype)



# revision 49
# speedup vs baseline: 1.5217x; 1.5217x over previous
"""Trainium2 distributed kernel for nn_CPAM_Module (CPAM attention block).

Math collapse (verified exact vs reference, ~2.6e-8 fro rel err in f64):
  te   = text_flat @ G_w.T + G_b                      (B, C)
  te_flat = te[:, :, None] * l  (rank-1 per batch)  =>
  proj_key / proj_value are rank-1 in n; energy[b,n,m] = a[b,n]*l[m] + const(n)
  softmax over m kills the const =>
  attn[b,n,m] = softmax_m(a[b,n] * l[m])
  a[b,n] = sum_c u[b,c] x[b,c,n] + b_q.kte[b],  u = M_u te,  M_u = W_q^T W_k
  S[b,n] = (sum_j l_j e^{l_j a}) / (sum_j e^{l_j a})
  out    = gamma*W_v te[b,:] * S[b,n] + gamma*b_v + x

Sharding: contraction (TXT=153600) split 8 ways for the big G matmul;
ReduceScatter of te (bf16) hands each core its 32 batches; epilogue is
batch-parallel. x is fully prefetched into SBUF behind the G-weight
stream on the same DMA queue, so HBM stays busy end-to-end; the
per-batch attention epilogue runs a[b,:] / broadcast / den-num on the
PE (small matmuls) with only exp + output eviction on DVE/Act/Pool.
"""

import sys

sys.path.insert(0, "/opt/trn_rl_repo")

import numpy as np
import ml_dtypes

from concourse import bass, bacc, mybir, tile
from concourse.bass_utils import run_bass_kernel_spmd

F32 = mybir.dt.float32
BF16 = mybir.dt.bfloat16
FP8 = mybir.dt.float8e4
GW_SCALE = 256.0
MU_SCALE = 64.0
AF = mybir.ActivationFunctionType
ALU = mybir.AluOpType

N_CORES = 8
B, C, H, W = 256, 1024, 14, 14
N = H * W  # 196
C8 = 128
TXT = 150 * 1024
KSH = TXT // N_CORES  # 19200 txt-contraction shard per core
NK = KSH // 128  # 150 k-tiles
BL = B // N_CORES  # 32 local batches
CT = C // 128  # 8 c tiles
JT = 98  # j-tile (196 = 2*98)
GRP = 4  # batches per x/out DMA group
NG = BL // GRP  # 8 groups
WAVE = 4


def build(gamma: float, skip_gb: bool, skip_bq: bool, skip_bv: bool, single: bool = False, repeat: int = 1, loop_n: int = 0, part: str = 'all'):
    # single=True builds a 1-core variant with the ReduceScatter replaced by a
    # local DMA (same bytes landing in te_rs) so TimelineSim can model it.
    nc = bacc.Bacc(
        "TRN2",
        target_bir_lowering=False,
        debug=False,
        num_devices=1 if single else N_CORES,
    )

    text_t = nc.dram_tensor("text_t", [128, NK * B], FP8, kind="ExternalInput")
    g_wt = nc.dram_tensor("g_wt", [128, NK * C], FP8, kind="ExternalInput")
    xs = nc.dram_tensor("xs", [128, BL * CT * N], BF16, kind="ExternalInput")
    mu_t = nc.dram_tensor("mu_t", [128, CT * C], FP8, kind="ExternalInput")
    gv_t = nc.dram_tensor("gv_t", [128, CT * C], FP8, kind="ExternalInput")
    lw = nc.dram_tensor("lw", [N, 2], BF16, kind="ExternalInput")
    l_col = nc.dram_tensor("l_col", [N, 1], F32, kind="ExternalInput")
    ident = nc.dram_tensor("ident", [BL, BL], BF16, kind="ExternalInput")
    g_b = nc.dram_tensor("g_b", [C8, CT], F32, kind="ExternalInput")
    vq_t = nc.dram_tensor("vq_t", [C8, CT], FP8, kind="ExternalInput")
    gbv = nc.dram_tensor("gbv", [C8, CT], F32, kind="ExternalInput")
    out = nc.dram_tensor("out", [128, BL * CT * N], BF16, kind="ExternalOutput")

    with tile.TileContext(nc) as tc:
        with (
            tc.tile_pool(name="const", bufs=1) as const,
            tc.tile_pool(name="xres", bufs=1) as xres,
            tc.tile_pool(name="dram", bufs=1, space="DRAM") as dram,
        ):
            # Constants on the Act HWDGE ring (SP ring is reserved for the
            # critical-path G stream + x prefetch).
            lw0 = const.tile([JT, 2], BF16, tag="lw0")
            lw1 = const.tile([JT, 2], BF16, tag="lw1")
            nc.scalar.dma_start(lw0[:], lw[0:JT, :])
            nc.scalar.dma_start(lw1[:], lw[JT : 2 * JT, :])
            lc0 = const.tile([JT, 1], F32, tag="lc0")
            lc1 = const.tile([JT, 1], F32, tag="lc1")
            nc.scalar.dma_start(lc0[:], l_col[0:JT, :])
            nc.scalar.dma_start(lc1[:], l_col[JT : 2 * JT, :])
            id_sb = const.tile([BL, BL], BF16, tag="id")
            nc.scalar.dma_start(id_sb[:], ident[:, :])
            mut_sb = const.tile([128, CT, C], FP8, tag="mut")
            nc.scalar.dma_start(mut_sb[:].opt(), mu_t[:, :])
            gvt_sb = const.tile([128, CT, C], FP8, tag="gvt")
            nc.scalar.dma_start(gvt_sb[:].opt(), gv_t[:, :])
            if not skip_gb:
                gb_sb = const.tile([C8, CT], F32, tag="gb")
                nc.scalar.dma_start(gb_sb[:], g_b[:, :])
            if not skip_bq:
                vq_sb = const.tile([C8, CT], FP8, tag="vq")
                nc.scalar.dma_start(vq_sb[:], vq_t[:, :])
            if not skip_bv:
                gbv_sb = const.tile([C8, CT], F32, tag="gbv")
                nc.scalar.dma_start(gbv_sb[:], gbv[:, :])
            ones128 = const.tile([1, C8], BF16, tag="ones128")
            nc.vector.memset(ones128[:], 1.0)

            # Resident x: one tile per 4-batch group so epilogue reads only
            # depend on their own group's DMA. Loaded on the SP ring BEHIND
            # the G stream (emitted after the g-loop below).
            xg = [
                xres.tile([128, GRP, CT, N], BF16, tag=f"xg{g}", name=f"xg{g}")
                for g in range(NG)
            ]

            te_full = dram.tile([B, C], BF16)
            te_rs = dram.tile([BL, C], BF16)

            if loop_n:
                assert single, "hardware loop timing mode is single-core only"
                loop_cm = tc.For_i(0, loop_n, 1)
                loop_cm.__enter__()
            for _rep in range(repeat):
                if part in ("all", "g"):
                    # ---- Phase 1: G matmul, te_partial[b, c] over local txt shard ----
                    with (
                        tc.tile_pool(name=f"gpsum{_rep}", bufs=4, space="PSUM") as gp,
                        tc.tile_pool(name=f"tl{_rep}", bufs=3) as tlp,
                        tc.tile_pool(name=f"gw{_rep}", bufs=3) as gwp,
                        tc.tile_pool(name=f"tesb{_rep}", bufs=2) as tesb,
                    ):
                        pt = [
                            [gp.tile([128, 512], F32, tag="gp", name=f"gp{_rep}_{m}{n2}") for n2 in range(2)]
                            for m in range(2)
                        ]
                        KB = 10  # k-tiles per DMA batch (150 = 15 * 10)
                        NPAIR = NK // 2
                        for g in range(NK // KB):
                            tl_t = tlp.tile([128, KB, B], FP8, tag="tl")
                            nc.sync.dma_start(tl_t[:].opt(), text_t[:, g * KB * B : (g + 1) * KB * B])
                            gw_t = gwp.tile([128, KB, C], FP8, tag="gw")
                            nc.sync.dma_start(gw_t[:].opt(), g_wt[:, g * KB * C : (g + 1) * KB * C])
                            for f in range(0, KB, 2):
                                j = (g * KB + f) // 2  # pair index
                                for m in range(2):
                                    for n2 in range(2):
                                        nc.tensor.matmul(
                                            pt[m][n2][:],
                                            tl_t[:, f : f + 2, m * 128 : (m + 1) * 128],
                                            gw_t[:, f : f + 2, n2 * 512 : (n2 + 1) * 512],
                                            start=(j == 0),
                                            stop=(j == NPAIR - 1),
                                            perf_mode=mybir.MatmulPerfMode.DoubleRow,
                                        )
                        # te eviction: split the 4 psum->sbuf muls across DVE
                        # and Act so the tail is short; evict DMAs go on the
                        # SP ring so they beat the x prefetch to the wire.
                        for m in range(2):
                            for n2 in range(2):
                                ev = tesb.tile([128, 512], BF16, tag="tesb")
                                if (m + n2) % 2 == 0:
                                    nc.vector.tensor_scalar_mul(ev[:], pt[m][n2][:], 1.0 / GW_SCALE)
                                else:
                                    nc.scalar.mul(ev[:], pt[m][n2][:], 1.0 / GW_SCALE)
                                nc.sync.dma_start(
                                    te_full[m * 128 : (m + 1) * 128, n2 * 512 : (n2 + 1) * 512],
                                    ev[:],
                                )

                    # ---- Phase 2: ReduceScatter -> local te (32, 1024) bf16 ----
                    if single:
                        nc.sync.dma_start(te_rs[:, :], te_full[0:BL, :])
                    else:
                        nc.gpsimd.collective_compute(
                            "ReduceScatter",
                            ALU.add,
                            replica_groups=[list(range(N_CORES))],
                            ins=[te_full.opt()],
                            outs=[te_rs.opt()],
                        )

                if part in ("all", "epi"):
                    # ---- Phase 3: teT via PE transpose, then uT / gvteT ----
                    te_sb = const.tile([BL, C], BF16, tag="te")
                    nc.sync.dma_start(te_sb[:], te_rs[:, :])
                    # x prefetch rides the Act ring (dep-free issues — they
                    # never block Act's sequencer) with a wait-hint so the
                    # scheduler doesn't start them before the te chain; the
                    # SP ring stays free for the out stream after te.
                    with tc.tile_wait_until(0.082, enable=(part == "all")):
                        for g in range(NG):
                            nc.scalar.dma_start(
                                xg[g][:].opt(),
                                xs[:, g * GRP * CT * N : (g + 1) * GRP * CT * N],
                            )
                    teT_sb = const.tile([128, CT, BL], FP8, tag="teT")
                    uT_sb = const.tile([128, CT, BL], BF16, tag="uT")
                    gvteT_sb = const.tile([128, CT, BL], F32, tag="gvteT")
                    bqd_row = const.tile([1, BL], F32, tag="bqd") if not skip_bq else None

                    with (
                        tc.tile_pool(name=f"ppsum{_rep}", bufs=2, space="PSUM") as pp,
                    ):
                        for t in range(CT):
                            tr_ps = pp.tile([128, BL], BF16, tag="tr", name=f"tr{_rep}_{t}")
                            nc.tensor.transpose(
                                tr_ps[:], te_sb[:, t * 128 : (t + 1) * 128], id_sb[:]
                            )
                            if skip_gb:
                                nc.vector.tensor_copy(teT_sb[:, t, :], tr_ps[:])
                            else:
                                nc.vector.tensor_scalar_add(
                                    teT_sb[:, t, :], tr_ps[:], gb_sb[:, t : t + 1]
                                )
                        # uT (c', b) = sum_c M_u^T[c, c'] teT[c, b]
                        for mt in range(CT):
                            u_ps = pp.tile([128, BL], F32, tag="u", name=f"u{_rep}_{mt}")
                            for kt in range(CT):
                                nc.tensor.matmul(
                                    u_ps[:],
                                    mut_sb[:, kt, mt * 128 : (mt + 1) * 128],
                                    teT_sb[:, kt, :],
                                    start=(kt == 0),
                                    stop=(kt == CT - 1),
                                )
                            nc.vector.tensor_scalar_mul(uT_sb[:, mt, :], u_ps[:], 1.0 / MU_SCALE)
                        # gvteT (c', b) = sum_c (gamma W_v)^T[c, c'] teT[c, b]
                        for mt in range(CT):
                            v_ps = pp.tile([128, BL], F32, tag="v", name=f"v{_rep}_{mt}")
                            for kt in range(CT):
                                nc.tensor.matmul(
                                    v_ps[:],
                                    gvt_sb[:, kt, mt * 128 : (mt + 1) * 128],
                                    teT_sb[:, kt, :],
                                    start=(kt == 0),
                                    stop=(kt == CT - 1),
                                )
                            nc.scalar.mul(gvteT_sb[:, mt, :], v_ps[:], 1.0 / MU_SCALE)
                        # bqd[b] = sum_c (W_k^T b_q)[c] teT[c, b]
                        if not skip_bq:
                            bq_ps = pp.tile([1, BL], F32, tag="bq", name=f"bq{_rep}")
                            for kt in range(CT):
                                nc.tensor.matmul(
                                    bq_ps[:],
                                    vq_sb[:, kt : kt + 1],
                                    teT_sb[:, kt, :],
                                    start=(kt == 0),
                                    stop=(kt == CT - 1),
                                )
                            nc.vector.tensor_scalar_mul(bqd_row[:], bq_ps[:], 1.0 / MU_SCALE)

                    # ---- Phase 4: attention epilogue, batch-PAIR granular ----
                    # Per pair: a-broadcast / exp / S-broadcast span both
                    # batches as [*, 2N] tiles (halving op count), and the
                    # S-broadcast is evicted once to SBUF bf16 so the 16
                    # output evictions read SBUF at 16-bit rate, not PSUM.
                    NP = BL // 2
                    with (
                        tc.tile_pool(name=f"op{_rep}", bufs=3) as op,
                        tc.tile_pool(name=f"esb{_rep}", bufs=3) as ep,
                        tc.tile_pool(name=f"small{_rep}", bufs=4) as sm,
                        tc.tile_pool(name=f"assb{_rep}", bufs=3) as asp,
                        tc.tile_pool(name=f"ps_pa{_rep}", bufs=2, space="PSUM") as ps_pa,
                        tc.tile_pool(name=f"ps_ab{_rep}", bufs=2, space="PSUM") as ps_ab,
                        tc.tile_pool(name=f"ps_dn{_rep}", bufs=2, space="PSUM") as ps_dn,
                        tc.tile_pool(name=f"ps_as{_rep}", bufs=2, space="PSUM") as ps_as,
                    ):
                        st_a, st_a2, st_ab, st_e, st_s, st_as = {}, {}, {}, {}, {}, {}

                        def stage_a(p):
                            # a_i[1, n] = sum_t uT[:, t, b].x[:, t, n] on PE;
                            # pair packed at partition rows 0 / 32 of one bank.
                            pa = ps_pa.tile([33, N], F32, tag="pa", name=f"pa{_rep}_{p}")
                            for i in (0, 1):
                                b = 2 * p + i
                                g, gi = divmod(b, GRP)
                                for t in range(CT):
                                    nc.tensor.matmul(
                                        pa[32 * i : 32 * i + 1, :],
                                        uT_sb[:, t, b : b + 1],
                                        xg[g][:, gi, t, :],
                                        start=(t == 0),
                                        stop=(t == CT - 1),
                                    )
                            st_a[p] = pa

                        def stage_b1(p):
                            # evict both a rows to one column-packed SBUF row
                            pa = st_a.pop(p)
                            a2 = sm.tile([1, 2, N], BF16, tag="a2", name=f"a2{_rep}_{p}")
                            for i in (0, 1):
                                if skip_bq:
                                    nc.vector.tensor_copy(a2[:, i, :], pa[32 * i : 32 * i + 1, :])
                                else:
                                    nc.vector.tensor_scalar_add(
                                        a2[:, i, :], pa[32 * i : 32 * i + 1, :],
                                        bqd_row[0:1, 2 * p + i : 2 * p + i + 1],
                                    )
                            st_a2[p] = a2

                        def stage_b2(p):
                            # broadcast both a rows over partitions (PE)
                            a2 = st_a2.pop(p)
                            ab = ps_ab.tile([128, 2, N], F32, tag="ab", name=f"ab{_rep}_{p}")
                            nc.tensor.matmul(ab[:], ones128[:], a2[:], start=True, stop=True)
                            st_ab[p] = ab

                        def stage_b3(p):
                            # exp halves (Act), both batches per op
                            ab = st_ab.pop(p)
                            e2 = ep.tile([JT, 2, 2, N], BF16, tag="e", name=f"e{_rep}_{p}")
                            nc.scalar.activation(e2[:, 0, :, :], ab[0:JT, :, :], AF.Exp, scale=lc0[:])
                            nc.scalar.activation(e2[:, 1, :, :], ab[0:JT, :, :], AF.Exp, scale=lc1[:])
                            st_e[p] = e2

                        def stage_c(p):
                            # den/num matmuls + S = num/den (DVE divide)
                            e2 = st_e.pop(p)
                            s2 = sm.tile([1, 2, N], BF16, tag="s2", name=f"s2{_rep}_{p}")
                            for i in (0, 1):
                                den_ps = ps_dn.tile([1, N], F32, tag="dn", name=f"den{_rep}_{p}_{i}")
                                num_ps = ps_dn.tile([1, N], F32, tag="dn", name=f"num{_rep}_{p}_{i}")
                                nc.tensor.matmul(den_ps[:], lw0[:, 0:1], e2[:, 0, i, :], start=True, stop=False)
                                nc.tensor.matmul(den_ps[:], lw1[:, 0:1], e2[:, 1, i, :], start=False, stop=True)
                                nc.tensor.matmul(num_ps[:], lw0[:, 1:2], e2[:, 0, i, :], start=True, stop=False)
                                nc.tensor.matmul(num_ps[:], lw1[:, 1:2], e2[:, 1, i, :], start=False, stop=True)
                                # an op may read only one PSUM input, and DVE
                                # has no divide: reciprocal then multiply
                                dinv = sm.tile([1, N], F32, tag="dinv", name=f"dinv{_rep}_{p}_{i}")
                                nc.vector.reciprocal(dinv[:], den_ps[:])
                                nc.vector.tensor_tensor(
                                    s2[:, i, :], num_ps[:], dinv[:], ALU.mult
                                )
                            st_s[p] = s2

                        def stage_d1(p):
                            # broadcast S over partitions (PE), evict to bf16
                            # SBUF once so all evictions read 16-bit SBUF
                            s2 = st_s.pop(p)
                            as_ps = ps_as.tile([128, 2, N], F32, tag="as", name=f"as{_rep}_{p}")
                            nc.tensor.matmul(as_ps[:], ones128[:], s2[:], start=True, stop=True)
                            as_sb = asp.tile([128, 2, N], BF16, tag="assb", name=f"assb{_rep}_{p}")
                            nc.scalar.copy(as_sb[:], as_ps[:])
                            st_as[p] = as_sb

                        def stage_d2(p):
                            # evict out = gvte*S + x for both batches
                            as_sb = st_as.pop(p)
                            for i in (0, 1):
                                b = 2 * p + i
                                g, gi = divmod(b, GRP)
                                ob = op.tile([128, CT, N], BF16, tag="ob", name=f"ob{_rep}_{b}")
                                ndve = 5 if i == 0 else 4
                                for t in range(ndve):
                                    nc.vector.scalar_tensor_tensor(
                                        ob[:, t, :],
                                        as_sb[:, i, :],
                                        gvteT_sb[:, t, b : b + 1],
                                        xg[g][:, gi, t, :],
                                        ALU.mult,
                                        ALU.add,
                                    )
                                for t in range(ndve, CT):
                                    nc.scalar.activation(
                                        ob[:, t, :], as_sb[:, i, :], AF.Copy,
                                        scale=gvteT_sb[:, t, b : b + 1],
                                    )
                                nc.gpsimd.tensor_add(
                                    ob[:, ndve:CT, :], ob[:, ndve:CT, :], xg[g][:, gi, ndve:CT, :]
                                )
                                if not skip_bv:
                                    for t in range(CT):
                                        nc.vector.tensor_scalar_add(
                                            ob[:, t, :], ob[:, t, :], gbv_sb[:, t : t + 1]
                                        )
                                nc.sync.dma_start(
                                    out[:, b * CT * N : (b + 1) * CT * N],
                                    ob[:].opt(),
                                )

                        # pair-granular software pipeline; per-engine queues
                        # see oldest-stage work first to avoid convoys.
                        def emit(stage, p):
                            if 0 <= p < NP:
                                stage(p)

                        for k in range(NP + 3):
                            emit(stage_b1, k - 1)   # DVE a evicts
                            emit(stage_d1, k - 3)   # PE as-broadcast + DVE evict
                            emit(stage_c, k - 2)    # PE dn + DVE divides
                            emit(stage_b2, k - 1)   # PE ab-broadcast
                            emit(stage_d2, k - 3)   # DVE STT / Act copies / Pool adds / out DMA
                            emit(stage_b3, k - 1)   # Act exps
                            emit(stage_a, k)        # PE a-matmuls
            if loop_n:
                loop_cm.__exit__(None, None, None)

    nc.compile()
    return nc


def _prep_inputs(inputs):
    """Host-side sharding. Returns in_maps for the 8 cores."""
    x = np.ascontiguousarray(inputs["x"], dtype=np.float32).reshape(B, C, N)
    text = np.ascontiguousarray(inputs["text_embed"], dtype=np.float32).reshape(B, -1)
    G_w = np.asarray(inputs["G_w"], dtype=np.float32)
    l = np.asarray(inputs["l"], dtype=np.float32).reshape(1, N)
    W_q = np.asarray(inputs["W_q"], dtype=np.float32)
    W_k = np.asarray(inputs["W_k"], dtype=np.float32)
    W_v = np.asarray(inputs["W_v"], dtype=np.float32)
    b_v = np.asarray(inputs["b_v"], dtype=np.float32)
    b_q = np.asarray(inputs["b_q"], dtype=np.float32)
    G_b = np.asarray(inputs["G_b"], dtype=np.float32)
    gamma = float(np.asarray(inputs["gamma"]).reshape(-1)[0])

    bf = ml_dtypes.bfloat16
    f8 = ml_dtypes.float8_e4m3

    def pretile(a, p=128):
        # (T*p, F) -> (p, T*F): partition-major tiling for contiguous DMA
        tp, f = a.shape
        t = tp // p
        return np.ascontiguousarray(a.reshape(t, p, f).transpose(1, 0, 2).reshape(p, t * f))

    M_u = W_q.T @ W_k  # (C, C): u = M_u te
    mu_t = pretile((np.ascontiguousarray(M_u.T) * 64.0).astype(f8))
    gv_t = pretile((np.ascontiguousarray((gamma * W_v).T) * 64.0).astype(f8))
    lw = np.stack([np.ones(N, np.float32), l[0]], axis=1)  # (196, 2)
    l_colv = np.ascontiguousarray(l.reshape(N, 1))
    ident = np.eye(BL, dtype=np.float32)
    g_b_t = np.ascontiguousarray(G_b.reshape(CT, C8).T)  # (128, 8)
    vq = W_k.T @ b_q  # (C,): bqd[b] = vq . te[b]
    vq_t = np.ascontiguousarray(vq.reshape(CT, C8).T)
    gbv = np.ascontiguousarray((gamma * b_v).reshape(CT, C8).T)

    in_maps = []
    for i in range(N_CORES):
        sl = slice(i * KSH, (i + 1) * KSH)
        # x: (BL, C, N) -> [p, b, t, n] partition-major
        xi = (
            x[i * BL : (i + 1) * BL]
            .reshape(BL, CT, 128, N)
            .transpose(2, 0, 1, 3)
            .reshape(128, BL * CT * N)
        )
        in_maps.append(
            {
                "text_t": pretile(np.ascontiguousarray(text[:, sl].T).astype(f8)),
                "g_wt": pretile((np.ascontiguousarray(G_w[:, sl].T) * GW_SCALE).astype(f8)),
                "xs": np.ascontiguousarray(xi).astype(bf),
                "mu_t": mu_t,
                "gv_t": gv_t,
                "lw": lw.astype(bf),
                "l_col": l_colv,
                "ident": ident.astype(bf),
                "g_b": g_b_t,
                "vq_t": (vq_t * 64.0).astype(f8),
                "gbv": gbv,
            }
        )
    meta = {
        "gamma": gamma,
        "skip_gb": not np.any(G_b),
        "skip_bq": not np.any(b_q),
        "skip_bv": not np.any(b_v),
    }
    return in_maps, meta


def _run(inputs, trace=False, repeat=1):
    in_maps, meta = _prep_inputs(inputs)
    nc = build(meta["gamma"], meta["skip_gb"], meta["skip_bq"], meta["skip_bv"], repeat=repeat)
    res = run_bass_kernel_spmd(nc, in_maps, core_ids=list(range(N_CORES)), trace=trace)
    outs = [
        res.results[i]["out"]
        .astype(np.float32)
        .reshape(128, BL, CT, N)
        .transpose(1, 2, 0, 3)
        .reshape(BL, C, N)
        for i in range(N_CORES)
    ]
    full = np.concatenate(outs, axis=0).reshape(B, C, H, W)
    return full, res


def kernel(**inputs) -> np.ndarray:
    full, _ = _run(inputs, trace=False)
    return full


if __name__ == "__main__":
    import reference

    inputs = {k: np.asarray(v) for k, v in reference.setup_inputs().items()}
    got = kernel(**inputs)
    print("out shape:", got.shape, got.dtype)


# revision 51
# speedup vs baseline: 1.6005x; 1.0518x over previous
"""Trainium2 distributed kernel for nn_CPAM_Module (CPAM attention block).

Math collapse (verified exact vs reference, ~2.6e-8 fro rel err in f64):
  te   = text_flat @ G_w.T + G_b                      (B, C)
  te_flat = te[:, :, None] * l  (rank-1 per batch)  =>
  proj_key / proj_value are rank-1 in n; energy[b,n,m] = a[b,n]*l[m] + const(n)
  softmax over m kills the const =>
  attn[b,n,m] = softmax_m(a[b,n] * l[m])
  a[b,n] = sum_c u[b,c] x[b,c,n] + b_q.kte[b],  u = M_u te,  M_u = W_q^T W_k
  S[b,n] = (sum_j l_j e^{l_j a}) / (sum_j e^{l_j a})
  out    = gamma*W_v te[b,:] * S[b,n] + gamma*b_v + x

Sharding: contraction (TXT=153600) split 8 ways for the big G matmul;
ReduceScatter of te (bf16) hands each core its 32 batches; epilogue is
batch-parallel. x is fully prefetched into SBUF behind the G-weight
stream on the same DMA queue, so HBM stays busy end-to-end; the
per-batch attention epilogue runs a[b,:] / broadcast / den-num on the
PE (small matmuls) with only exp + output eviction on DVE/Act/Pool.
"""

import sys

sys.path.insert(0, "/opt/trn_rl_repo")

import numpy as np
import ml_dtypes

from concourse import bass, bacc, mybir, tile
from concourse.bass_utils import run_bass_kernel_spmd

F32 = mybir.dt.float32
BF16 = mybir.dt.bfloat16
FP8 = mybir.dt.float8e4
GW_SCALE = 256.0
MU_SCALE = 64.0
AF = mybir.ActivationFunctionType
ALU = mybir.AluOpType

N_CORES = 8
B, C, H, W = 256, 1024, 14, 14
N = H * W  # 196
C8 = 128
TXT = 150 * 1024
KSH = TXT // N_CORES  # 19200 txt-contraction shard per core
NK = KSH // 128  # 150 k-tiles
BL = B // N_CORES  # 32 local batches
CT = C // 128  # 8 c tiles
JT = 98  # j-tile (196 = 2*98)
GRP = 4  # batches per x/out DMA group
NG = BL // GRP  # 8 groups
WAVE = 4


def build(gamma: float, skip_gb: bool, skip_bq: bool, skip_bv: bool, single: bool = False, repeat: int = 1, loop_n: int = 0, part: str = 'all'):
    # single=True builds a 1-core variant with the ReduceScatter replaced by a
    # local DMA (same bytes landing in te_rs) so TimelineSim can model it.
    nc = bacc.Bacc(
        "TRN2",
        target_bir_lowering=False,
        debug=False,
        num_devices=1 if single else N_CORES,
    )

    text_t = nc.dram_tensor("text_t", [128, NK * B], FP8, kind="ExternalInput")
    g_wt = nc.dram_tensor("g_wt", [128, NK * C], FP8, kind="ExternalInput")
    xs = nc.dram_tensor("xs", [128, BL * CT * N], BF16, kind="ExternalInput")
    mu_t = nc.dram_tensor("mu_t", [128, CT * C], FP8, kind="ExternalInput")
    gv_t = nc.dram_tensor("gv_t", [128, CT * C], FP8, kind="ExternalInput")
    lw = nc.dram_tensor("lw", [N, 2], BF16, kind="ExternalInput")
    l_col = nc.dram_tensor("l_col", [N, 1], F32, kind="ExternalInput")
    ident = nc.dram_tensor("ident", [BL, BL], BF16, kind="ExternalInput")
    g_b = nc.dram_tensor("g_b", [C8, CT], F32, kind="ExternalInput")
    vq_t = nc.dram_tensor("vq_t", [C8, CT], FP8, kind="ExternalInput")
    gbv = nc.dram_tensor("gbv", [C8, CT], F32, kind="ExternalInput")
    out = nc.dram_tensor("out", [128, BL * CT * N], BF16, kind="ExternalOutput")

    with tile.TileContext(nc) as tc:
        with (
            tc.tile_pool(name="const", bufs=1) as const,
            tc.tile_pool(name="xres", bufs=1) as xres,
            tc.tile_pool(name="dram", bufs=1, space="DRAM") as dram,
        ):
            # Constants on the Act HWDGE ring (SP ring is reserved for the
            # critical-path G stream + x prefetch).
            lw0 = const.tile([JT, 2], BF16, tag="lw0")
            lw1 = const.tile([JT, 2], BF16, tag="lw1")
            nc.scalar.dma_start(lw0[:], lw[0:JT, :])
            nc.scalar.dma_start(lw1[:], lw[JT : 2 * JT, :])
            lc0 = const.tile([JT, 1], F32, tag="lc0")
            lc1 = const.tile([JT, 1], F32, tag="lc1")
            nc.scalar.dma_start(lc0[:], l_col[0:JT, :])
            nc.scalar.dma_start(lc1[:], l_col[JT : 2 * JT, :])
            id_sb = const.tile([BL, BL], BF16, tag="id")
            nc.scalar.dma_start(id_sb[:], ident[:, :])
            mut_sb = const.tile([128, CT, C], FP8, tag="mut")
            nc.scalar.dma_start(mut_sb[:].opt(), mu_t[:, :])
            gvt_sb = const.tile([128, CT, C], FP8, tag="gvt")
            nc.scalar.dma_start(gvt_sb[:].opt(), gv_t[:, :])
            if not skip_gb:
                gb_sb = const.tile([C8, CT], F32, tag="gb")
                nc.scalar.dma_start(gb_sb[:], g_b[:, :])
            if not skip_bq:
                vq_sb = const.tile([C8, CT], FP8, tag="vq")
                nc.scalar.dma_start(vq_sb[:], vq_t[:, :])
            if not skip_bv:
                gbv_sb = const.tile([C8, CT], F32, tag="gbv")
                nc.scalar.dma_start(gbv_sb[:], gbv[:, :])
            ones128 = const.tile([1, C8], BF16, tag="ones128")
            nc.vector.memset(ones128[:], 1.0)

            # Resident x: one tile per 4-batch group so epilogue reads only
            # depend on their own group's DMA. Loaded on the SP ring BEHIND
            # the G stream (emitted after the g-loop below).
            xg = [
                xres.tile([128, GRP, CT, N], BF16, tag=f"xg{g}", name=f"xg{g}")
                for g in range(NG)
            ]

            te_full = dram.tile([B, C], BF16)
            te_rs = dram.tile([BL, C], BF16)

            if loop_n:
                assert single, "hardware loop timing mode is single-core only"
                loop_cm = tc.For_i(0, loop_n, 1)
                loop_cm.__enter__()
            for _rep in range(repeat):
                if part in ("all", "g"):
                    # ---- Phase 1: G matmul, te_partial[b, c] over local txt shard ----
                    with (
                        tc.tile_pool(name=f"gpsum{_rep}", bufs=4, space="PSUM") as gp,
                        tc.tile_pool(name=f"tl{_rep}", bufs=3) as tlp,
                        tc.tile_pool(name=f"gw{_rep}", bufs=3) as gwp,
                        tc.tile_pool(name=f"tesb{_rep}", bufs=2) as tesb,
                    ):
                        pt = [
                            [gp.tile([128, 512], F32, tag="gp", name=f"gp{_rep}_{m}{n2}") for n2 in range(2)]
                            for m in range(2)
                        ]
                        KB = 10  # k-tiles per DMA batch (150 = 15 * 10)
                        NPAIR = NK // 2
                        for g in range(NK // KB):
                            tl_t = tlp.tile([128, KB, B], FP8, tag="tl")
                            nc.sync.dma_start(tl_t[:].opt(), text_t[:, g * KB * B : (g + 1) * KB * B])
                            gw_t = gwp.tile([128, KB, C], FP8, tag="gw")
                            nc.sync.dma_start(gw_t[:].opt(), g_wt[:, g * KB * C : (g + 1) * KB * C])
                            for f in range(0, KB, 2):
                                j = (g * KB + f) // 2  # pair index
                                for m in range(2):
                                    for n2 in range(2):
                                        nc.tensor.matmul(
                                            pt[m][n2][:],
                                            tl_t[:, f : f + 2, m * 128 : (m + 1) * 128],
                                            gw_t[:, f : f + 2, n2 * 512 : (n2 + 1) * 512],
                                            start=(j == 0),
                                            stop=(j == NPAIR - 1),
                                            perf_mode=mybir.MatmulPerfMode.DoubleRow,
                                        )
                        # te eviction: split the 4 psum->sbuf muls across DVE
                        # and Act so the tail is short; evict DMAs go on the
                        # SP ring so they beat the x prefetch to the wire.
                        for m in range(2):
                            for n2 in range(2):
                                ev = tesb.tile([128, 512], BF16, tag="tesb")
                                if (m + n2) % 2 == 0:
                                    nc.vector.tensor_scalar_mul(ev[:], pt[m][n2][:], 1.0 / GW_SCALE)
                                else:
                                    nc.scalar.mul(ev[:], pt[m][n2][:], 1.0 / GW_SCALE)
                                nc.sync.dma_start(
                                    te_full[m * 128 : (m + 1) * 128, n2 * 512 : (n2 + 1) * 512],
                                    ev[:],
                                )

                    # ---- Phase 2: ReduceScatter -> local te (32, 1024) bf16 ----
                    if single:
                        nc.sync.dma_start(te_rs[:, :], te_full[0:BL, :])
                    else:
                        nc.gpsimd.collective_compute(
                            "ReduceScatter",
                            ALU.add,
                            replica_groups=[list(range(N_CORES))],
                            ins=[te_full.opt()],
                            outs=[te_rs.opt()],
                        )

                if part in ("all", "epi"):
                    # ---- Phase 3: teT via PE transpose, then uT / gvteT ----
                    te_sb = const.tile([BL, C], BF16, tag="te")
                    nc.sync.dma_start(te_sb[:], te_rs[:, :])
                    # x prefetch rides the SP ring behind the te chain. The
                    # wait-hint keeps the list scheduler from hoisting these
                    # ahead of the te evicts (they have no data deps).
                    with tc.tile_wait_until(0.085, enable=(part == "all")):
                        for g in range(NG):
                            nc.sync.dma_start(
                                xg[g][:].opt(),
                                xs[:, g * GRP * CT * N : (g + 1) * GRP * CT * N],
                            )
                    teT_sb = const.tile([128, CT, BL], FP8, tag="teT")
                    uT_sb = const.tile([128, CT, BL], BF16, tag="uT")
                    gvteT_sb = const.tile([128, CT, BL], F32, tag="gvteT")
                    bqd_row = const.tile([1, BL], F32, tag="bqd") if not skip_bq else None

                    with (
                        tc.tile_pool(name=f"ppsum{_rep}", bufs=2, space="PSUM") as pp,
                    ):
                        for t in range(CT):
                            tr_ps = pp.tile([128, BL], BF16, tag="tr", name=f"tr{_rep}_{t}")
                            nc.tensor.transpose(
                                tr_ps[:], te_sb[:, t * 128 : (t + 1) * 128], id_sb[:]
                            )
                            if skip_gb:
                                nc.vector.tensor_copy(teT_sb[:, t, :], tr_ps[:])
                            else:
                                nc.vector.tensor_scalar_add(
                                    teT_sb[:, t, :], tr_ps[:], gb_sb[:, t : t + 1]
                                )
                        # uT (c', b) = sum_c M_u^T[c, c'] teT[c, b]
                        for mt in range(CT):
                            u_ps = pp.tile([128, BL], F32, tag="u", name=f"u{_rep}_{mt}")
                            for kt in range(CT):
                                nc.tensor.matmul(
                                    u_ps[:],
                                    mut_sb[:, kt, mt * 128 : (mt + 1) * 128],
                                    teT_sb[:, kt, :],
                                    start=(kt == 0),
                                    stop=(kt == CT - 1),
                                )
                            nc.vector.tensor_scalar_mul(uT_sb[:, mt, :], u_ps[:], 1.0 / MU_SCALE)
                        # gvteT (c', b) = sum_c (gamma W_v)^T[c, c'] teT[c, b]
                        for mt in range(CT):
                            v_ps = pp.tile([128, BL], F32, tag="v", name=f"v{_rep}_{mt}")
                            for kt in range(CT):
                                nc.tensor.matmul(
                                    v_ps[:],
                                    gvt_sb[:, kt, mt * 128 : (mt + 1) * 128],
                                    teT_sb[:, kt, :],
                                    start=(kt == 0),
                                    stop=(kt == CT - 1),
                                )
                            nc.scalar.mul(gvteT_sb[:, mt, :], v_ps[:], 1.0 / MU_SCALE)
                        # bqd[b] = sum_c (W_k^T b_q)[c] teT[c, b]
                        if not skip_bq:
                            bq_ps = pp.tile([1, BL], F32, tag="bq", name=f"bq{_rep}")
                            for kt in range(CT):
                                nc.tensor.matmul(
                                    bq_ps[:],
                                    vq_sb[:, kt : kt + 1],
                                    teT_sb[:, kt, :],
                                    start=(kt == 0),
                                    stop=(kt == CT - 1),
                                )
                            nc.vector.tensor_scalar_mul(bqd_row[:], bq_ps[:], 1.0 / MU_SCALE)

                    # ---- Phase 4: attention epilogue, batch-PAIR granular ----
                    # Per pair: a-broadcast / exp / S-broadcast span both
                    # batches as [*, 2N] tiles (halving op count), and the
                    # S-broadcast is evicted once to SBUF bf16 so the 16
                    # output evictions read SBUF at 16-bit rate, not PSUM.
                    NP = BL // 2
                    with (
                        tc.tile_pool(name=f"op{_rep}", bufs=3) as op,
                        tc.tile_pool(name=f"esb{_rep}", bufs=3) as ep,
                        tc.tile_pool(name=f"small{_rep}", bufs=4) as sm,
                        tc.tile_pool(name=f"assb{_rep}", bufs=3) as asp,
                        tc.tile_pool(name=f"ps_pa{_rep}", bufs=2, space="PSUM") as ps_pa,
                        tc.tile_pool(name=f"ps_ab{_rep}", bufs=2, space="PSUM") as ps_ab,
                        tc.tile_pool(name=f"ps_dn{_rep}", bufs=2, space="PSUM") as ps_dn,
                        tc.tile_pool(name=f"ps_as{_rep}", bufs=2, space="PSUM") as ps_as,
                    ):
                        st_a, st_a2, st_ab, st_e, st_s, st_as = {}, {}, {}, {}, {}, {}

                        def stage_a(p):
                            # a_i[1, n] = sum_t uT[:, t, b].x[:, t, n] on PE;
                            # pair packed at partition rows 0 / 32 of one bank.
                            pa = ps_pa.tile([33, N], F32, tag="pa", name=f"pa{_rep}_{p}")
                            for i in (0, 1):
                                b = 2 * p + i
                                g, gi = divmod(b, GRP)
                                for t in range(CT):
                                    nc.tensor.matmul(
                                        pa[32 * i : 32 * i + 1, :],
                                        uT_sb[:, t, b : b + 1],
                                        xg[g][:, gi, t, :],
                                        start=(t == 0),
                                        stop=(t == CT - 1),
                                    )
                            st_a[p] = pa

                        def stage_b1(p):
                            # evict both a rows to one column-packed SBUF row
                            pa = st_a.pop(p)
                            a2 = sm.tile([1, 2, N], BF16, tag="a2", name=f"a2{_rep}_{p}")
                            for i in (0, 1):
                                if skip_bq:
                                    nc.vector.tensor_copy(a2[:, i, :], pa[32 * i : 32 * i + 1, :])
                                else:
                                    nc.vector.tensor_scalar_add(
                                        a2[:, i, :], pa[32 * i : 32 * i + 1, :],
                                        bqd_row[0:1, 2 * p + i : 2 * p + i + 1],
                                    )
                            st_a2[p] = a2

                        def stage_b2(p):
                            # broadcast both a rows over partitions (PE)
                            a2 = st_a2.pop(p)
                            ab = ps_ab.tile([128, 2, N], F32, tag="ab", name=f"ab{_rep}_{p}")
                            nc.tensor.matmul(ab[:], ones128[:], a2[:], start=True, stop=True)
                            st_ab[p] = ab

                        def stage_b3(p):
                            # exp halves (Act), both batches per op
                            ab = st_ab.pop(p)
                            e2 = ep.tile([JT, 2, 2, N], BF16, tag="e", name=f"e{_rep}_{p}")
                            nc.scalar.activation(e2[:, 0, :, :], ab[0:JT, :, :], AF.Exp, scale=lc0[:])
                            nc.scalar.activation(e2[:, 1, :, :], ab[0:JT, :, :], AF.Exp, scale=lc1[:])
                            st_e[p] = e2

                        def stage_c(p):
                            # den/num matmuls + S = num/den (DVE divide)
                            e2 = st_e.pop(p)
                            s2 = sm.tile([1, 2, N], BF16, tag="s2", name=f"s2{_rep}_{p}")
                            for i in (0, 1):
                                den_ps = ps_dn.tile([1, N], F32, tag="dn", name=f"den{_rep}_{p}_{i}")
                                num_ps = ps_dn.tile([1, N], F32, tag="dn", name=f"num{_rep}_{p}_{i}")
                                nc.tensor.matmul(den_ps[:], lw0[:, 0:1], e2[:, 0, i, :], start=True, stop=False)
                                nc.tensor.matmul(den_ps[:], lw1[:, 0:1], e2[:, 1, i, :], start=False, stop=True)
                                nc.tensor.matmul(num_ps[:], lw0[:, 1:2], e2[:, 0, i, :], start=True, stop=False)
                                nc.tensor.matmul(num_ps[:], lw1[:, 1:2], e2[:, 1, i, :], start=False, stop=True)
                                # an op may read only one PSUM input, and DVE
                                # has no divide: reciprocal then multiply
                                dinv = sm.tile([1, N], F32, tag="dinv", name=f"dinv{_rep}_{p}_{i}")
                                nc.vector.reciprocal(dinv[:], den_ps[:])
                                nc.vector.tensor_tensor(
                                    s2[:, i, :], num_ps[:], dinv[:], ALU.mult
                                )
                            st_s[p] = s2

                        def stage_d1(p):
                            # broadcast S over partitions (PE), evict to bf16
                            # SBUF once so all evictions read 16-bit SBUF
                            s2 = st_s.pop(p)
                            as_ps = ps_as.tile([128, 2, N], F32, tag="as", name=f"as{_rep}_{p}")
                            nc.tensor.matmul(as_ps[:], ones128[:], s2[:], start=True, stop=True)
                            as_sb = asp.tile([128, 2, N], BF16, tag="assb", name=f"assb{_rep}_{p}")
                            nc.scalar.copy(as_sb[:], as_ps[:])
                            st_as[p] = as_sb

                        def stage_d2(p):
                            # evict out = gvte*S + x for both batches
                            as_sb = st_as.pop(p)
                            for i in (0, 1):
                                b = 2 * p + i
                                g, gi = divmod(b, GRP)
                                ob = op.tile([128, CT, N], BF16, tag="ob", name=f"ob{_rep}_{b}")
                                ndve = 5 if i == 0 else 4
                                for t in range(ndve):
                                    nc.vector.scalar_tensor_tensor(
                                        ob[:, t, :],
                                        as_sb[:, i, :],
                                        gvteT_sb[:, t, b : b + 1],
                                        xg[g][:, gi, t, :],
                                        ALU.mult,
                                        ALU.add,
                                    )
                                for t in range(ndve, CT):
                                    nc.scalar.activation(
                                        ob[:, t, :], as_sb[:, i, :], AF.Copy,
                                        scale=gvteT_sb[:, t, b : b + 1],
                                    )
                                nc.gpsimd.tensor_add(
                                    ob[:, ndve:CT, :], ob[:, ndve:CT, :], xg[g][:, gi, ndve:CT, :]
                                )
                                if not skip_bv:
                                    for t in range(CT):
                                        nc.vector.tensor_scalar_add(
                                            ob[:, t, :], ob[:, t, :], gbv_sb[:, t : t + 1]
                                        )
                                nc.sync.dma_start(
                                    out[:, b * CT * N : (b + 1) * CT * N],
                                    ob[:].opt(),
                                )

                        # pair-granular software pipeline; per-engine queues
                        # see oldest-stage work first to avoid convoys.
                        def emit(stage, p):
                            if 0 <= p < NP:
                                stage(p)

                        for k in range(NP + 3):
                            emit(stage_b1, k - 1)   # DVE a evicts
                            emit(stage_d1, k - 3)   # PE as-broadcast + DVE evict
                            emit(stage_c, k - 2)    # PE dn + DVE divides
                            emit(stage_b2, k - 1)   # PE ab-broadcast
                            emit(stage_d2, k - 3)   # DVE STT / Act copies / Pool adds / out DMA
                            emit(stage_b3, k - 1)   # Act exps
                            emit(stage_a, k)        # PE a-matmuls
            if loop_n:
                loop_cm.__exit__(None, None, None)

    nc.compile()
    return nc


def _prep_inputs(inputs):
    """Host-side sharding. Returns in_maps for the 8 cores."""
    x = np.ascontiguousarray(inputs["x"], dtype=np.float32).reshape(B, C, N)
    text = np.ascontiguousarray(inputs["text_embed"], dtype=np.float32).reshape(B, -1)
    G_w = np.asarray(inputs["G_w"], dtype=np.float32)
    l = np.asarray(inputs["l"], dtype=np.float32).reshape(1, N)
    W_q = np.asarray(inputs["W_q"], dtype=np.float32)
    W_k = np.asarray(inputs["W_k"], dtype=np.float32)
    W_v = np.asarray(inputs["W_v"], dtype=np.float32)
    b_v = np.asarray(inputs["b_v"], dtype=np.float32)
    b_q = np.asarray(inputs["b_q"], dtype=np.float32)
    G_b = np.asarray(inputs["G_b"], dtype=np.float32)
    gamma = float(np.asarray(inputs["gamma"]).reshape(-1)[0])

    bf = ml_dtypes.bfloat16
    f8 = ml_dtypes.float8_e4m3

    def pretile(a, p=128):
        # (T*p, F) -> (p, T*F): partition-major tiling for contiguous DMA
        tp, f = a.shape
        t = tp // p
        return np.ascontiguousarray(a.reshape(t, p, f).transpose(1, 0, 2).reshape(p, t * f))

    M_u = W_q.T @ W_k  # (C, C): u = M_u te
    mu_t = pretile((np.ascontiguousarray(M_u.T) * 64.0).astype(f8))
    gv_t = pretile((np.ascontiguousarray((gamma * W_v).T) * 64.0).astype(f8))
    lw = np.stack([np.ones(N, np.float32), l[0]], axis=1)  # (196, 2)
    l_colv = np.ascontiguousarray(l.reshape(N, 1))
    ident = np.eye(BL, dtype=np.float32)
    g_b_t = np.ascontiguousarray(G_b.reshape(CT, C8).T)  # (128, 8)
    vq = W_k.T @ b_q  # (C,): bqd[b] = vq . te[b]
    vq_t = np.ascontiguousarray(vq.reshape(CT, C8).T)
    gbv = np.ascontiguousarray((gamma * b_v).reshape(CT, C8).T)

    in_maps = []
    for i in range(N_CORES):
        sl = slice(i * KSH, (i + 1) * KSH)
        # x: (BL, C, N) -> [p, b, t, n] partition-major
        xi = (
            x[i * BL : (i + 1) * BL]
            .reshape(BL, CT, 128, N)
            .transpose(2, 0, 1, 3)
            .reshape(128, BL * CT * N)
        )
        in_maps.append(
            {
                "text_t": pretile(np.ascontiguousarray(text[:, sl].T).astype(f8)),
                "g_wt": pretile((np.ascontiguousarray(G_w[:, sl].T) * GW_SCALE).astype(f8)),
                "xs": np.ascontiguousarray(xi).astype(bf),
                "mu_t": mu_t,
                "gv_t": gv_t,
                "lw": lw.astype(bf),
                "l_col": l_colv,
                "ident": ident.astype(bf),
                "g_b": g_b_t,
                "vq_t": (vq_t * 64.0).astype(f8),
                "gbv": gbv,
            }
        )
    meta = {
        "gamma": gamma,
        "skip_gb": not np.any(G_b),
        "skip_bq": not np.any(b_q),
        "skip_bv": not np.any(b_v),
    }
    return in_maps, meta


def _run(inputs, trace=False, repeat=1):
    in_maps, meta = _prep_inputs(inputs)
    nc = build(meta["gamma"], meta["skip_gb"], meta["skip_bq"], meta["skip_bv"], repeat=repeat)
    res = run_bass_kernel_spmd(nc, in_maps, core_ids=list(range(N_CORES)), trace=trace)
    outs = [
        res.results[i]["out"]
        .astype(np.float32)
        .reshape(128, BL, CT, N)
        .transpose(1, 2, 0, 3)
        .reshape(BL, C, N)
        for i in range(N_CORES)
    ]
    full = np.concatenate(outs, axis=0).reshape(B, C, H, W)
    return full, res


def kernel(**inputs) -> np.ndarray:
    full, _ = _run(inputs, trace=False)
    return full


if __name__ == "__main__":
    import reference

    inputs = {k: np.asarray(v) for k, v in reference.setup_inputs().items()}
    got = kernel(**inputs)
    print("out shape:", got.shape, got.dtype)
